# revision 49
# baseline (speedup 1.0000x reference)
"""NemotronH Mamba2 mixer on 8 Trainium2 cores (Bass/Tile).

Sharding: tensor-parallel over heads/groups. Core c owns group c =
16 heads (= 1024 gate/x channels, 128 B + 128 C state channels, 16 dt).
in_proj rows and out_proj columns are sharded accordingly; out_proj is
row-parallel over the contraction, partials are combined on the host.

v3: both projections run as fp8(e4m3) DoubleRow matmuls (K=256 per
pass, 0.5 cyc/row) with a 3-pass hi/lo correction that reuses tensors:
  PSUM = W8 x8 + W8 xlo + Wr x8  ~=  64 W x   (W8=e4m3(64W),
  Wr=e4m3(64W-W8), x8=e4m3(x), xlo=e4m3(x-x8))
The 1/64 is folded into consumers (conv weights, silu scale, dt
activation scale, host-side gather). Gate tiles are produced in
transposed [seq, chan] orientation straight into silu (drops the
per-chunk gate transposes); per-head cs broadcast is a single fp32r
matmul per head (free dim 256); out_proj moving operand is converted
to fp8 hi/lo on-chip right after the norm transposes.
"""

import numpy as np
import ml_dtypes

import concourse.bass as bass
import concourse.mybir as mybir
from concourse import bacc
from concourse.tile import TileContext
from concourse.bass_utils import run_bass_kernel_spmd

F32 = mybir.dt.float32
F32R = mybir.dt.float32r
BF16 = mybir.dt.bfloat16
F8 = mybir.dt.float8e4
AF = mybir.ActivationFunctionType
ALU = mybir.AluOpType
NPBF16 = ml_dtypes.bfloat16
NPF8 = ml_dtypes.float8_e4m3

# Model dims
H_SIZE = 4096
NH = 128
HD = 64
SS = 128
KCONV = 4
NG = 8
CHUNK = 128
INTER = NH * HD                 # 8192
CONV_DIM = INTER + 2 * NG * SS  # 10240
PROJ = INTER + CONV_DIM + NH    # 18560
DT_MIN, DT_MAX = 0.001, 100.0
EPS = 1e-5
GROUP = INTER // NG             # 1024

# Sharding / tiling
N_CORES = 8
S = 2048
HL = NH // N_CORES              # 16 local heads
CLOC = HL * HD                  # 1024 local gate/x channels
NSB = 4                         # seq superblocks
SB = S // NSB                   # 512
NCPB = SB // CHUNK              # 4 chunks per superblock
NKB = H_SIZE // 256             # 16 fp8 K-blocks for in_proj
NT = 19                         # w1 tiles: 0-7 x, 8 B, 9 C, 10 dt, 11-18 gate
NK2 = CLOC // 128               # 8 k-tiles for out_proj
NB2 = CLOC // 256               # 4 fp8 K-blocks for out_proj
NM2 = H_SIZE // 128             # 32 m-tiles for out_proj
WS = 64.0                       # fp8 weight prescale

_CACHE = {}


def r32(ap):
    return ap.bitcast(F32R)


def ap3(t, col, s1, n1, s2, n2):
    """3-dim AP into tile t at free offset col: [part][s1,n1][s2,n2]."""
    base = t[:, col:col + 1]
    return bass.AP(tensor=t.tensor, offset=base.offset,
                   ap=[base.ap[0], [s1, n1], [s2, n2]])


def _steer_act_tables():
    """Make the table-load placement pass choose the joint exp+ln table.

    The pass picks the first act-func-set containing each required
    function; by default Exp resolves to exp_and_others and Ln to
    natural_log, which thrashes a table load on every Exp<->Ln switch
    (dt pipeline, per-chunk rmsnorm). Hiding exp/ln from the
    single-function sets steers both to natural_log_exp_and_others.
    Set ids (= act_info.json indices) are untouched, so the runtime
    tables walrus emits stay valid.
    """
    if getattr(bacc, "_act_tables_steered", False):
        return
    orig = bacc.get_activation_tables

    def patched(arch):
        t = dict(orig(arch))
        for name in list(t):
            funcs = set(t[name])
            if name == "exp_and_others":
                funcs.discard(AF.Exp)
            if name == "natural_log":
                funcs.discard(AF.Ln)
            t[name] = funcs
        return t

    bacc.get_activation_tables = patched
    bacc._act_tables_steered = True


def build_nc():
    _steer_act_tables()
    nc = bacc.Bacc(None, target_bir_lowering=False)

    # fp8 hidden, quarter-split: [sb, q, 128, 4*1024] where block b=4q+bq
    # holds [2 ktiles x 512 seq] at col bq*1024+i*512+s
    hid8 = nc.declare_dram_parameter("hid8", [NSB, 4, 128, 4096], F8,
                                     isOutput=False)
    hidlo = nc.declare_dram_parameter("hidlo", [NSB, 4, 128, 4096], F8,
                                      isOutput=False)
    # in_proj weights per tile: [t, 128, b*256 + i*128 + m] fp8
    w18 = nc.declare_dram_parameter("w18", [NT, 128, NKB * 256], F8,
                                    isOutput=False)
    w1r = nc.declare_dram_parameter("w1r", [NT, 128, NKB * 256], F8,
                                    isOutput=False)
    # out_proj weights, groups of 4 m-tiles: [G, 128, g*1024+B*256+i*128+m]
    w28 = nc.declare_dram_parameter("w28", [NM2 // 4, 128, 4096], F8,
                                    isOutput=False)
    w2r = nc.declare_dram_parameter("w2r", [NM2 // 4, 128, 4096], F8,
                                    isOutput=False)
    convw = nc.declare_dram_parameter("convw", [128, 10 * KCONV], F32,
                                      isOutput=False)
    convb = nc.declare_dram_parameter("convb", [128, 10], F32, isOutput=False)
    dtbias = nc.declare_dram_parameter("dtbias", [HL, 1], F32, isOutput=False)
    acol = nc.declare_dram_parameter("acol", [HL, 1], F32, isOutput=False)
    dbc = nc.declare_dram_parameter("dbc", [128, HL], F32, isOutput=False)
    idf = nc.declare_dram_parameter("idf", [128, 128], F32, isOutput=False)
    idb = nc.declare_dram_parameter("idb", [128, 128], BF16, isOutput=False)
    trim = nc.declare_dram_parameter("trim", [128, 128], BF16, isOutput=False)
    # output partials at WS scale (host divides by WS)
    outp = nc.declare_dram_parameter("outp", [NM2, NSB, 128, SB], BF16,
                                     isOutput=True)

    with TileContext(nc) as tc:
        with tc.tile_pool(name="const", bufs=1) as cp:
            idf_sb = cp.tile([128, 128], F32, tag="idf")
            idb_sb = cp.tile([128, 128], BF16, tag="idb")
            trim_sb = cp.tile([128, 128], BF16, tag="trim")
            cw_sb = cp.tile([128, 10 * KCONV], F32, tag="cw")
            cb_sb = cp.tile([128, 10], F32, tag="cb")
            dtb_sb = cp.tile([HL, 1], F32, tag="dtb")
            a_sb = cp.tile([HL, 1], F32, tag="acol")
            dbc_sb = cp.tile([128, HL], F32, tag="dbc")
            ones16 = cp.tile([HL, 1], F32, tag="ones16")
            zcol = cp.tile([128, 1], F32, tag="zcol")
            st_sb = cp.tile([128, HL * HD], F32R, tag="state")
            stT = cp.tile([128, HL * HD], BF16, tag="stateb")
            nc.vector.memset(ones16[:], 1.0)
            nc.vector.memset(zcol[:], 0.0)

            # const DMAs are issued inside _main_phase after sb0's
            # critical-path fp8 loads (9 small DMAs would otherwise delay
            # the first in_proj tile by several us)
            def load_consts():
                nc.sync.dma_start(out=dtb_sb[:], in_=dtbias[:])
                nc.sync.dma_start(out=a_sb[:], in_=acol[:])
                nc.sync.dma_start(out=idf_sb[:], in_=idf[:])
                nc.sync.dma_start(out=cw_sb[:], in_=convw[:])
                nc.sync.dma_start(out=cb_sb[:], in_=convb[:])
                nc.sync.dma_start(out=idb_sb[:], in_=idb[:])
                nc.sync.dma_start(out=trim_sb[:], in_=trim[:])
                nc.sync.dma_start(out=dbc_sb[:], in_=dbc[:])

            _main_phase(nc, tc, load_consts, hid8, hidlo, w18, w1r, w28, w2r,
                        outp, idf_sb, idb_sb, trim_sb, cw_sb, cb_sb,
                        dtb_sb, a_sb, dbc_sb, ones16, zcol, st_sb, stT)

    nc.compile()
    return nc


def _main_phase(nc, tc, load_consts, hid8, hidlo, w18, w1r, w28, w2r, outp,
                idf_sb, idb_sb, trim_sb, cw_sb, cb_sb,
                dtb_sb, a_sb, dbc_sb, ones16, zcol, st_sb, stT):
    with tc.tile_pool(name="hid", bufs=1) as hidp, \
         tc.tile_pool(name="w1", bufs=3) as w1p, \
         tc.tile_pool(name="w2", bufs=2) as w2p, \
         tc.tile_pool(name="stage", bufs=1) as sgp, \
         tc.tile_pool(name="qst", bufs=2) as qstp, \
         tc.tile_pool(name="conv32", bufs=2) as cvp, \
         tc.tile_pool(name="pair", bufs=1) as prp, \
         tc.tile_pool(name="seg", bufs=2) as segp, \
         tc.tile_pool(name="ch", bufs=2) as chp, \
         tc.tile_pool(name="ch1", bufs=1) as ch1p, \
         tc.tile_pool(name="oev", bufs=3) as oevp, \
         tc.tile_pool(name="acc", bufs=2, space="PSUM") as accp, \
         tc.tile_pool(name="psY", bufs=2, space="PSUM") as psY, \
         tc.tile_pool(name="psPB", bufs=2, space="PSUM") as psPB, \
         tc.tile_pool(name="psT", bufs=2, space="PSUM") as psT:

        # conv input staging: 10 channel tiles (8 x, 1 B, 1 C), 3 halo + SB
        # values are at WS scale; conv weights are pre-divided by WS.
        ccat = sgp.tile([128, 10 * (SB + 3)], F32, tag="ccat")
        for t in range(10):
            nc.vector.memset(ccat[:, t * (SB + 3):t * (SB + 3) + 3], 0.0)

        pending_out = []

        w2cache = {}

        def load_w2_pair(G, name="w2"):
            t8 = w2p.tile([128, 4096], F8, tag="w28", name=name + "8")
            tr = w2p.tile([128, 4096], F8, tag="w2r", name=name + "r")
            nc.sync.dma_start(out=t8[:], in_=w28[G])
            nc.sync.dma_start(out=tr[:], in_=w2r[G])
            return (t8, tr)

        def emit_outproj(m, qpair, sbq, pool=None, tag="acc"):
            q8t, qlot = qpair
            G, g = m // 4, m % 4
            if w2cache.get("G") != G:
                pair = (w2cache.get("pref")
                        if w2cache.get("prefG") == G else None)
                if pair is None:
                    pair = load_w2_pair(G)
                w2cache["G"] = G
                w2cache["t"] = pair
                w2cache["pref"] = None
            w2t8, w2tr = w2cache["t"]
            if g == 0 and G + 1 < NM2 // 4 and w2cache.get("prefG") != G + 1:
                w2cache["prefG"] = G + 1
                w2cache["pref"] = load_w2_pair(G + 1, name="w2pref")
            acc = (pool or accp).tile([128, SB], F32, tag=tag)
            for h in range(2):
                out = acc[:, h * 256:(h + 1) * 256]
                n = 0
                for B in range(NB2):
                    st8 = ap3(w2t8, g * 1024 + B * 256, 128, 2, 1, 128)
                    str_ = ap3(w2tr, g * 1024 + B * 256, 128, 2, 1, 128)
                    mv8 = ap3(q8t, (2 * B) * SB + h * 256, SB, 2, 1, 256)
                    mvlo = ap3(qlot, (2 * B) * SB + h * 256, SB, 2, 1, 256)
                    for sta, mv in ((st8, mv8), (st8, mvlo), (str_, mv8)):
                        nc.tensor.matmul(
                            out, sta, mv,
                            start=(n == 0), stop=(n == 3 * NB2 - 1),
                            perf_mode=mybir.MatmulPerfMode.DoubleRow)
                        n += 1
            ev = oevp.tile([128, SB], BF16, tag="oev")
            nc.scalar.copy(ev[:], acc[:])
            nc.sync.dma_start(out=outp[m, sbq], in_=ev[:])

        for sb in range(NSB):
            # DMA order follows pass-major consumption: W8, x8 quarters,
            # Wr, then xlo quarters
            w1pre8 = w1p.tile([128, NKB * 256], F8, tag="w18", name="w1pre8")
            w1prer = w1p.tile([128, NKB * 256], F8, tag="w1r", name="w1prer")
            nc.sync.dma_start(out=w1pre8[:], in_=w18[10])
            h8q, hloq = [], []
            for q in range(4):
                t8 = hidp.tile([128, 4096], F8, tag=f"hid8{q}",
                               name=f"hid8{q}")
                nc.sync.dma_start(out=t8[:], in_=hid8[sb, q])
                h8q.append(t8)
            nc.sync.dma_start(out=w1prer[:], in_=w1r[10])
            for q in range(4):
                tlo = hidp.tile([128, 4096], F8, tag=f"hidlo{q}",
                                name=f"hidlo{q}")
                nc.sync.dma_start(out=tlo[:], in_=hidlo[sb, q])
                hloq.append(tlo)
            if sb == 0:
                load_consts()

            dtraw = sgp.tile([HL, SB], F32, tag="dtraw")
            silg_sb = sgp.tile([128, NCPB * CLOC], BF16, tag="silg_sb")

            # halo copies must read previous superblock before overwrite
            if sb > 0:
                for t in range(10):
                    base = t * (SB + 3)
                    nc.vector.tensor_copy(
                        ccat[:, base:base + 3], ccat[:, base + SB:base + SB + 3])

            # pass structure per K-block: (W8,x8), (W8,xlo), (Wr,x8)
            def emit_stationary_tile(t, dtraw=dtraw):
                if t == 10:
                    w8t, wrt = w1pre8, w1prer
                else:
                    w8t = w1p.tile([128, NKB * 256], F8, tag="w18")
                    wrt = w1p.tile([128, NKB * 256], F8, tag="w1r")
                    nc.sync.dma_start(out=w8t[:], in_=w18[t])
                    nc.sync.dma_start(out=wrt[:], in_=w1r[t])
                acc = accp.tile([128, SB], F32, tag="acc")
                for h in range(2):
                    out = acc[:, h * 256:(h + 1) * 256]
                    n = 0
                    # pass-major: (W8,x8), (Wr,x8), (W8,xlo) so the first
                    # passes only need the hi-tensor DMAs
                    for which in range(3):
                        for b in range(NKB):
                            q, qc = b // 4, (b % 4) * 1024 + h * 256
                            if which == 0:
                                sta = ap3(w8t, b * 256, 128, 2, 1, 128)
                                mv = ap3(h8q[q], qc, 512, 2, 1, 256)
                            elif which == 1:
                                sta = ap3(wrt, b * 256, 128, 2, 1, 128)
                                mv = ap3(h8q[q], qc, 512, 2, 1, 256)
                            else:
                                sta = ap3(w8t, b * 256, 128, 2, 1, 128)
                                mv = ap3(hloq[q], qc, 512, 2, 1, 256)
                            nc.tensor.matmul(
                                out, sta, mv,
                                start=(n == 0), stop=(n == 3 * NKB - 1),
                                perf_mode=mybir.MatmulPerfMode.DoubleRow)
                            n += 1
                if t < 10:
                    base = t * (SB + 3)
                    nc.scalar.copy(ccat[:, base + 3:base + 3 + SB], acc[:])
                else:
                    nc.scalar.copy(dtraw[:, :], acc[:HL, :])

            def emit_gate_tile(tg, silg_sb=silg_sb):
                w8t = w1p.tile([128, NKB * 256], F8, tag="w18")
                wrt = w1p.tile([128, NKB * 256], F8, tag="w1r")
                nc.sync.dma_start(out=w8t[:], in_=w18[11 + tg])
                nc.sync.dma_start(out=wrt[:], in_=w1r[11 + tg])
                acc = accp.tile([128, SB], F32, tag="acc")
                for st in range(NCPB):
                    out = acc[:, st * 128:(st + 1) * 128]
                    n = 0
                    for b in range(NKB):
                        q, qc = b // 4, (b % 4) * 1024 + st * 128
                        lh8 = ap3(h8q[q], qc, 512, 2, 1, 128)
                        lhlo = ap3(hloq[q], qc, 512, 2, 1, 128)
                        mv8 = ap3(w8t, b * 256, 128, 2, 1, 128)
                        mvr = ap3(wrt, b * 256, 128, 2, 1, 128)
                        for sta, mv in ((lh8, mv8), (lhlo, mv8), (lh8, mvr)):
                            nc.tensor.matmul(
                                out, sta, mv,
                                start=(n == 0), stop=(n == 3 * NKB - 1),
                                perf_mode=mybir.MatmulPerfMode.DoubleRow)
                            n += 1
                # silu straight out of PSUM into [seq, chan] slab
                dst = ap3(silg_sb, tg * 128, CLOC, NCPB, 1, 128)
                src = ap3(acc, 0, 128, NCPB, 1, 128)
                nc.scalar.activation(dst, src, AF.Silu, scale=1.0 / WS)

            def emit_dt_pipeline():
                az = sgp.tile([HL, SB], F32, tag="az")
                dtsp = dtraw  # in-place: relu(z)+ln1p overwrites raw dt
                nc.scalar.activation(az[:], dtraw[:], AF.Abs,
                                     bias=dtb_sb[:, 0:1], scale=1.0 / WS)
                nc.scalar.activation(az[:], az[:], AF.Exp, scale=-1.0)
                nc.vector.tensor_scalar(az[:], az[:], 1.0, None, ALU.add)
                nc.scalar.activation(az[:], az[:], AF.Ln)
                nc.scalar.activation(dtsp[:], dtraw[:], AF.Relu,
                                     bias=dtb_sb[:, 0:1], scale=1.0 / WS)
                nc.vector.tensor_tensor(dtsp[:], dtsp[:], az[:], ALU.add)
                nc.vector.tensor_scalar(dtsp[:], dtsp[:], DT_MIN, DT_MAX,
                                        ALU.max, ALU.min)
                dA = az  # az dead, reuse
                nc.vector.tensor_scalar(dA[:], dtsp[:], a_sb[:, 0:1], None,
                                        ALU.mult)
                for cl in range(NCPB):
                    ones_b = bass.AP(tensor=ones16.tensor,
                                     offset=ones16[:].offset,
                                     ap=[ones16[:].ap[0], [0, CHUNK]])
                    nc.vector.tensor_tensor_scan(
                        cs[:, cl * CHUNK:(cl + 1) * CHUNK],
                        ones_b, dA[:, cl * CHUNK:(cl + 1) * CHUNK],
                        0.0, ALU.mult, ALU.add)
                return dtsp

            cs = sgp.tile([HL, SB], F32, tag="cs")
            drain = list(pending_out)
            pending_out.clear()
            emit_stationary_tile(10)
            dtsp = emit_dt_pipeline()
            for u in [8, 9] + list(range(8)):
                emit_stationary_tile(u)

            # csT/dtT for all chunks: [128, cl*HL + h] / [128, (4+cl)*HL + h]
            pcs = psPB.tile([128, 2 * NCPB * HL], F32, tag="pb",
                            name="pcs")
            for cl in range(NCPB):
                nc.tensor.transpose(
                    pcs[:, cl * HL:(cl + 1) * HL],
                    cs[:, cl * CHUNK:(cl + 1) * CHUNK], idf_sb[:HL, :HL])
                nc.tensor.transpose(
                    pcs[:, (NCPB + cl) * HL:(NCPB + cl + 1) * HL],
                    dtsp[:, cl * CHUNK:(cl + 1) * CHUNK], idf_sb[:HL, :HL])
            csdtT = sgp.tile([128, 2 * NCPB * HL], F32, tag="csdtT")
            nc.scalar.copy(csdtT[:], pcs[:])
            negcsT = sgp.tile([128, NCPB * HL], F32, tag="negcsT")
            nc.vector.tensor_scalar(negcsT[:], csdtT[:, :NCPB * HL], -1.0,
                                    None, ALU.mult)

            # bf16 triple splits of cs per pair: the broadcast matmuls need
            # ~21 bits of cs mantissa (fp32r/tf32 is NOT enough: |cs| can be
            # ~2e4 while exp needs abs err << 0.01)
            splits = []
            for pr2 in range(NCPB // 2):
                p2sl = slice(pr2 * 2 * CHUNK, (pr2 + 1) * 2 * CHUNK)
                csh = sgp.tile([HL, 2 * CHUNK], BF16, tag="csh", bufs=2,
                               name=f"csh{pr2}")
                csm = sgp.tile([HL, 2 * CHUNK], BF16, tag="csm", bufs=2,
                               name=f"csm{pr2}")
                csl_ = sgp.tile([HL, 2 * CHUNK], BF16, tag="csl", bufs=2,
                               name=f"csl{pr2}")
                res = sgp.tile([HL, 2 * CHUNK], F32, tag="csres", bufs=2,
                               name=f"res{pr2}")
                nc.vector.tensor_copy(csh[:], cs[:, p2sl])
                nc.vector.tensor_tensor(res[:], cs[:, p2sl], csh[:],
                                        ALU.subtract)
                nc.vector.tensor_copy(csm[:], res[:])
                nc.vector.tensor_tensor(res[:], res[:], csm[:], ALU.subtract)
                nc.vector.tensor_copy(csl_[:], res[:])
                splits.append((csh, csm, csl_))

            # conv (DVE-heavy) emitted before gate tiles (PE-heavy) so the
            # two engines overlap; conv + gate silus share one table window
            xcs = sgp.tile([128, 8 * SB], BF16, tag="xcs")
            bcs = sgp.tile([128, SB], BF16, tag="bcs")
            ccs = sgp.tile([128, SB], BF16, tag="ccs")
            for t in [8, 9] + list(range(8)):
                base = t * (SB + 3)
                eng = nc.vector
                c32 = cvp.tile([128, SB], F32, tag="c32")
                eng.tensor_scalar(
                    c32[:], ccat[:, base:base + SB],
                    cw_sb[:, t * KCONV:t * KCONV + 1], cb_sb[:, t:t + 1],
                    ALU.mult, ALU.add)
                for j in range(1, KCONV):
                    eng.scalar_tensor_tensor(
                        c32[:], ccat[:, base + j:base + j + SB],
                        cw_sb[:, t * KCONV + j:t * KCONV + j + 1], c32[:],
                        ALU.mult, ALU.add)
                dst = (xcs[:, t * SB:(t + 1) * SB] if t < 8
                       else (bcs[:] if t == 8 else ccs[:]))
                nc.scalar.activation(dst, c32[:], AF.Silu)

            for tg in range(8):
                emit_gate_tile(tg)

            q8t = qstp.tile([128, NK2 * SB], F8, tag="q8")
            qlot = qstp.tile([128, NK2 * SB], F8, tag="qlo")
            ssum = sgp.tile([128, NCPB], F32, tag="ssum")

            # ---------------- SSD chunk pairs ----------------
            for pr in range(NCPB // 2):
                prsl = slice(pr * 2 * CHUNK, (pr + 1) * 2 * CHUNK)
                csh, csm, csl_ = splits[pr]
                # per-head cs broadcast: pb[p, j*256+l] = cs[h, pr*256+l]
                epb = prp.tile([128, HL * 2 * CHUNK], F32R, tag="epb")
                segs = [segp.tile([128, HL * CHUNK], F32R, tag="seg",
                                  name=f"seg{i}")
                        for i in range(2)]
                for hg in range(HL // 2):
                    # PE filler: the pb->seg->exp chain is Act/DVE-bound
                    if hg % 2 == 1 and drain:
                        emit_outproj(*drain.pop(0))
                    pb = psPB.tile([128, 512], F32, tag="pb")
                    for j in range(2):
                        h = 2 * hg + j
                        idcol = idb_sb[:HL, h:h + 1]
                        indh = bass.AP(tensor=idcol.tensor,
                                       offset=idcol.offset,
                                       ap=[[idcol.ap[0][0], HL], [0, 128]])
                        for si, spl in enumerate((csh, csm, csl_)):
                            nc.tensor.matmul(pb[:, j * 256:(j + 1) * 256],
                                             indh, spl[:],
                                             start=(si == 0), stop=(si == 2))
                    # seg[s, l] = min(cs[h,l] - cs[h,s], 0) per chunk
                    for lc in range(2):
                        cl = 2 * pr + lc
                        for j in range(2):
                            h = 2 * hg + j
                            nc.vector.scalar_tensor_tensor(
                                segs[lc][:, h * CHUNK:(h + 1) * CHUNK],
                                pb[:, j * 256 + lc * 128:
                                   j * 256 + (lc + 1) * 128],
                                negcsT[:, cl * HL + h:cl * HL + h + 1],
                                bass.AP(tensor=zcol.tensor,
                                        offset=zcol[:].offset,
                                        ap=[zcol[:].ap[0], [0, CHUNK]]),
                                ALU.add, ALU.min)
                    nc.scalar.activation(epb[:, hg * 512:(hg + 1) * 512],
                                         pb[:], AF.Exp)

                for lc in range(2):
                    cl = 2 * pr + lc
                    def filler(n, drain=drain):
                        for _ in range(min(n, len(drain))):
                            emit_outproj(*drain.pop(0))
                    _emit_chunk(nc, sb * NCPB + cl, cl, lc, silg_sb, xcs,
                                bcs, ccs,
                                csdtT, segs[lc], epb, q8t, qlot, ssum,
                                idf_sb, idb_sb, trim_sb, dbc_sb,
                                st_sb, stT, chp, ch1p, psY, psT, psPB,
                                filler)

            while drain:
                emit_outproj(*drain.pop(0))

            pending_out.extend((m, (q8t, qlot), sb) for m in range(NM2))

        # final drain: rotate across all psum pools so the ev-copy WAR
        # latency of one bank hides behind matmuls into another
        pools = [(accp, "acc"), (psY, "y"), (psPB, "pb")]
        i = 0
        while pending_out:
            pool, tag = pools[i % 3]
            i += 1
            emit_outproj(*pending_out.pop(0), pool=pool, tag=tag)


def _emit_chunk(nc, gc, cl, lc, silg_sb, xcs, bcs, ccs,
                csdtT, seg, epb, q8t, qlot, ssum,
                idf_sb, idb_sb, trim_sb, dbc_sb,
                st_sb, stT, chp, ch1p, psY, psT, psPB, filler):
    csl = slice(cl * CHUNK, (cl + 1) * CHUNK)

    # gate already in [seq, chan] with silu applied
    silg = silg_sb[:, cl * CLOC:(cl + 1) * CLOC]

    # scores = exp(seg) * (triu-in-[s,l] . gram); gram^T = B C^T in [s, l]
    gram_ps = psPB.tile([128, 128], F32, tag="pb", name="gram_ps")
    nc.tensor.matmul(gram_ps[:], bcs[:, csl], ccs[:, csl],
                     start=True, stop=True)
    gram = chp.tile([128, 128], F32, tag="gramm")
    nc.vector.tensor_tensor(gram[:], gram_ps[:], trim_sb[:], ALU.mult)

    # chunk-end decay per head: cend = exp(cs_end), decT = exp(cs_end - cs)
    # (both extracted BEFORE seg/epb are overwritten in place below)
    cend = chp.tile([128, HL], F32, tag="cend")
    ep1 = epb[:, (lc + 1) * CHUNK - 1:(lc + 1) * CHUNK]
    epb_end = bass.AP(tensor=epb.tensor, offset=ep1.offset,
                      ap=[ep1.ap[0], [2 * CHUNK, HL]])
    nc.vector.tensor_copy(cend[:], epb_end)
    decT = chp.tile([128, HL], F32, tag="decT")
    # seg column l=CHUNK-1 holds cs_end - cs[s] (<=0, min-clamp no-op there)
    sg1 = seg[:, CHUNK - 1:CHUNK]
    seg_end = bass.AP(tensor=seg.tensor, offset=sg1.offset,
                      ap=[sg1.ap[0], [CHUNK, HL]])
    nc.scalar.activation(decT[:], seg_end, AF.Exp)

    # scores = exp(seg) * gram -> bf16
    scores = chp.tile([128, HL * CHUNK], BF16, tag="scores", bufs=1)
    nc.scalar.activation(scores[:], seg[:], AF.Exp)
    s3 = scores[:].rearrange("p (h l) -> p h l", h=HL)
    gram_b = bass.AP(tensor=gram.tensor, offset=gram[:].offset,
                     ap=[gram[:].ap[0], [0, HL], [1, 128]])
    nc.vector.tensor_tensor(s3, s3, gram_b, ALU.mult)

    # e4 = exp(pb) * C (for Yoff) -> bf16
    e4 = chp.tile([128, HL * CHUNK], BF16, tag="e4", bufs=1)
    e4_3 = e4[:].rearrange("p (h l) -> p h l", h=HL)
    ep0 = epb[:, lc * CHUNK:lc * CHUNK + 1]
    epb_3 = bass.AP(tensor=epb.tensor, offset=ep0.offset,
                    ap=[ep0.ap[0], [2 * CHUNK, HL], [1, CHUNK]])
    cc0 = ccs[:, cl * CHUNK:cl * CHUNK + 1]
    ccs_b = bass.AP(tensor=ccs.tensor, offset=cc0.offset,
                    ap=[cc0.ap[0], [0, HL], [1, CHUNK]])
    nc.vector.tensor_tensor(e4_3, epb_3, ccs_b, ALU.mult)
    ddt = chp.tile([128, HL], F32, tag="ddt")
    nc.vector.tensor_tensor(ddt[:], csdtT[:, (NCPB + cl) * HL:
                                           (NCPB + cl + 1) * HL],
                            decT[:], ALU.mult)

    # x transpose -> xT (bf16), then xdt / xdd
    xT = ch1p.tile([128, CLOC], BF16, tag="xT")
    for hx in range(2):
        xps = psT.tile([128, 512], BF16, tag="trans", name=f"xps{hx}")
        for t in range(4):
            tt = hx * 4 + t
            nc.tensor.transpose(
                xps[:, t * 128:(t + 1) * 128],
                xcs[:, tt * SB + cl * CHUNK:tt * SB + (cl + 1) * CHUNK],
                idb_sb[:])
        nc.scalar.copy(xT[:, hx * 512:(hx + 1) * 512], xps[:])
    xdt = ch1p.tile([128, CLOC], BF16, tag="xdt")
    x3 = xT[:].rearrange("p (h j) -> p h j", h=HL)
    dt0 = csdtT[:, (NCPB + cl) * HL:(NCPB + cl) * HL + 1]
    dt_b = bass.AP(tensor=csdtT.tensor, offset=dt0.offset,
                   ap=[dt0.ap[0], [1, HL], [0, HD]])
    ddt_b = bass.AP(tensor=ddt.tensor, offset=ddt[:].offset,
                    ap=[ddt[:].ap[0], [1, HL], [0, HD]])
    nc.vector.tensor_tensor(xdt[:].rearrange("p (h j) -> p h j", h=HL),
                            x3, dt_b, ALU.mult)
    # ysb = x*D now (before xdd overwrites xT in place)
    dbc_b = bass.AP(tensor=dbc_sb.tensor, offset=dbc_sb[:].offset,
                    ap=[dbc_sb[:].ap[0], [1, HL], [0, HD]])
    ysb = ch1p.tile([128, CLOC], F32, tag="ysb")
    nc.vector.tensor_tensor(ysb[:].rearrange("p (h j) -> p h j", h=HL),
                            x3, dbc_b, ALU.mult)
    xdd = xT  # in place: x * ddt overwrites xT
    nc.vector.tensor_tensor(xdd[:].rearrange("p (h j) -> p h j", h=HL),
                            x3, ddt_b, ALU.mult)

    # B chunk transposed (bln) for state matmuls
    pbt = psPB.tile([128, 128], BF16, tag="pb", name="pbt")
    nc.tensor.transpose(pbt[:], bcs[:, csl], idb_sb[:])
    bln = chp.tile([128, 128], BF16, tag="bln")
    nc.scalar.copy(bln[:], pbt[:])

    filler(3)

    # Ydiag + Yoff accumulated per head (two matmuls per head)
    y_halves = []
    for half in range(2):
        y_ps = psY.tile([128, 512], F32, tag="y", name=f"y{half}")
        for hh in range(8):
            h = half * 8 + hh
            hs = slice(hh * HD, (hh + 1) * HD)
            nc.tensor.matmul(
                y_ps[:, hs], scores[:, h * CHUNK:(h + 1) * CHUNK],
                xdt[:, h * HD:(h + 1) * HD], start=True, stop=(gc == 0))
            if gc > 0:
                nc.tensor.matmul(
                    y_ps[:, hs],
                    e4[:, h * CHUNK:(h + 1) * CHUNK],
                    stT[:, h * HD:(h + 1) * HD], start=False, stop=True)
        y_halves.append(y_ps)

    # states for this chunk
    s_halves = []
    for half in range(2):
        s_ps = psY.tile([128, 512], F32, tag="y", name=f"s{half}")
        nc.tensor.matmul(
            s_ps[:], bln[:], xdd[:, half * 512:(half + 1) * 512],
            start=True, stop=True)
        s_halves.append(s_ps)

    filler(5)

    # y = (Ydiag + Yoff) + D*x, gated; squares accumulated for RMS
    for half in range(2):
        hsl = slice(half * 512, (half + 1) * 512)
        nc.vector.tensor_tensor(ysb[:, hsl], ysb[:, hsl],
                                y_halves[half][:], ALU.add)
    nc.vector.tensor_tensor(ysb[:], ysb[:], silg, ALU.mult)
    nc.scalar.activation(xdt[:], ysb[:], AF.Square,
                         accum_out=ssum[:, cl:cl + 1])

    # per-chunk group RMSNorm + transpose + fp8 hi/lo conversion
    lnm = chp.tile([128, 1], F32, tag="lnm")
    rstd = chp.tile([128, 1], F32, tag="rstd")
    nc.vector.tensor_scalar(lnm[:], ssum[:, cl:cl + 1], 1.0 / GROUP, EPS,
                            ALU.mult, ALU.add)
    nc.scalar.activation(lnm[:], lnm[:], AF.Ln)
    nc.scalar.activation(rstd[:], lnm[:], AF.Exp, scale=-0.5)
    normed = ch1p.tile([128, CLOC], BF16, tag="normed")
    nc.vector.tensor_scalar(normed[:], ysb[:], rstd[:, 0:1], None, ALU.mult)
    nps = psT.tile([128, CLOC], BF16, tag="trans")
    for t in range(NK2):
        nc.tensor.transpose(
            nps[:, t * 128:(t + 1) * 128],
            normed[:, t * 128:(t + 1) * 128], idb_sb[:])
    nsrc = nps[:].rearrange("p (t s) -> p t s", t=NK2)
    q8dst = ap3(q8t, cl * 128, SB, NK2, 1, 128)
    qlodst = ap3(qlot, cl * 128, SB, NK2, 1, 128)
    nc.scalar.copy(q8dst, nsrc)
    nc.vector.tensor_tensor(qlodst, nsrc, q8dst, ALU.subtract)

    # state update: st = st * exp(cs_end) + s  (first chunk: st = s)
    if gc == 0:
        for half in range(2):
            hsl = slice(half * 512, (half + 1) * 512)
            nc.vector.tensor_copy(st_sb[:, hsl], s_halves[half][:])
        nc.vector.tensor_copy(stT[:], st_sb[:])
    else:
        cend_b = bass.AP(tensor=cend.tensor, offset=cend[:].offset,
                         ap=[cend[:].ap[0], [1, HL], [0, HD]])
        st3 = st_sb[:].rearrange("p (h j) -> p h j", h=HL)
        nc.vector.tensor_tensor(st3, st3, cend_b, ALU.mult)
        for half in range(2):
            hsl = slice(half * 512, (half + 1) * 512)
            nc.vector.tensor_tensor(st_sb[:, hsl], st_sb[:, hsl],
                                    s_halves[half][:], ALU.add)
        nc.vector.tensor_copy(stT[:], st_sb[:])


def _q8(a):
    return np.asarray(a, NPF8)


def prepare_in_maps(hidden_states, in_proj_w, conv_w, conv_b, dt_bias, D,
                    norm_w, out_proj_w):
    hidT = np.ascontiguousarray(
        hidden_states.reshape(S, H_SIZE).T).astype(np.float32)
    x8 = _q8(hidT)
    xlo = _q8(hidT - x8.astype(np.float32))

    def hid_layout(a):
        # [k=4096, s=2048] -> [sb, q, p, bq*1024 + i*512 + s]
        return np.ascontiguousarray(
            a.reshape(4, 4, 2, 128, NSB, 512)
            .transpose(4, 0, 3, 1, 2, 5).reshape(NSB, 4, 128, 4096))

    idf = np.eye(128, dtype=np.float32)
    idb = np.eye(128).astype(NPBF16)
    # mask in [s, l]: keep l >= s
    trim = np.triu(np.ones((128, 128), np.float32)).astype(NPBF16)
    in_maps = []
    for c in range(N_CORES):
        gsl = slice(CLOC * c, CLOC * (c + 1))
        xsl = slice(INTER + CLOC * c, INTER + CLOC * (c + 1))
        bsl = slice(2 * INTER + SS * c, 2 * INTER + SS * (c + 1))
        cslc = slice(2 * INTER + NG * SS + SS * c,
                     2 * INTER + NG * SS + SS * (c + 1))
        dsl = slice(INTER + CONV_DIM + HL * c, INTER + CONV_DIM + HL * (c + 1))
        dt_rows = np.concatenate(
            [in_proj_w[dsl],
             np.zeros((128 - HL, H_SIZE), np.float32)], axis=0)
        # tiles 0-7 x, 8 B, 9 C, 10 dt, 11-18 gate
        w1 = np.concatenate([in_proj_w[xsl], in_proj_w[bsl], in_proj_w[cslc],
                             dt_rows, in_proj_w[gsl]], axis=0) * WS
        w1_8 = _q8(w1)
        w1_r = _q8(w1 - w1_8.astype(np.float32))

        def w1_layout(a):
            # [t*128+m, k=256b+128i+p] -> [t, p, b*256 + i*128 + m]
            return np.ascontiguousarray(
                a.reshape(NT, 128, NKB, 2, 128)
                .transpose(0, 4, 2, 3, 1).reshape(NT, 128, NKB * 256))

        w2 = (out_proj_w[:, gsl] * norm_w[gsl][None, :]) * WS
        w2_8 = _q8(w2)
        w2_r = _q8(w2 - w2_8.astype(np.float32))

        def w2_layout(a):
            # [(4G+g)*128+m, c=256B+128i+p] -> [G, p, g*1024+B*256+i*128+m]
            return np.ascontiguousarray(
                a.reshape(8, 4, 128, NB2, 2, 128)
                .transpose(0, 5, 1, 3, 4, 2).reshape(8, 128, 4096))

        conv_idx = np.concatenate([
            np.arange(CLOC * c, CLOC * (c + 1)),
            np.arange(INTER + SS * c, INTER + SS * (c + 1)),
            np.arange(INTER + NG * SS + SS * c,
                      INTER + NG * SS + SS * (c + 1))])
        cwl = conv_w[conv_idx, 0, :] / WS     # [1280, 4], WS folded
        cbl = conv_b[conv_idx]                # [1280]
        convw = np.ascontiguousarray(
            cwl.reshape(10, 128, KCONV).transpose(1, 0, 2)
            .reshape(128, 10 * KCONV)).astype(np.float32)
        convb = np.ascontiguousarray(
            cbl.reshape(10, 128).transpose(1, 0)).astype(np.float32)
        hsl = slice(HL * c, HL * (c + 1))
        acol = -(np.arange(HL * c + 1, HL * (c + 1) + 1, dtype=np.float32))
        in_maps.append({
            "hid8": hid_layout(x8),
            "hidlo": hid_layout(xlo),
            "w18": w1_layout(w1_8),
            "w1r": w1_layout(w1_r),
            "w28": w2_layout(w2_8),
            "w2r": w2_layout(w2_r),
            "convw": convw,
            "convb": convb,
            "dtbias": dt_bias[hsl].reshape(HL, 1).astype(np.float32),
            "acol": acol.reshape(HL, 1),
            "dbc": np.tile(D[hsl][None, :], (128, 1)).astype(np.float32),
            "idf": idf,
            "idb": idb,
            "trim": trim,
        })
    return in_maps


def get_nc():
    if "nc" not in _CACHE:
        _CACHE["nc"] = build_nc()
    return _CACHE["nc"]


def kernel(hidden_states, in_proj_w, conv_w, conv_b, dt_bias, D, norm_w,
           out_proj_w):
    nc = get_nc()
    in_maps = prepare_in_maps(
        np.asarray(hidden_states, np.float32),
        np.asarray(in_proj_w, np.float32),
        np.asarray(conv_w, np.float32), np.asarray(conv_b, np.float32),
        np.asarray(dt_bias, np.float32), np.asarray(D, np.float32),
        np.asarray(norm_w, np.float32), np.asarray(out_proj_w, np.float32))
    res = run_bass_kernel_spmd(nc, in_maps, list(range(N_CORES)))
    acc = np.zeros((H_SIZE, S), np.float64)
    for r in res.results:
        acc += np.asarray(r["outp"], np.float64).transpose(0, 2, 1, 3) \
                 .reshape(H_SIZE, S)
    return (acc / WS).T.astype(np.float32).reshape(1, S, H_SIZE)


# revision 57
# speedup vs baseline: 1.0180x; 1.0180x over previous
"""NemotronH Mamba2 mixer on 8 Trainium2 cores (Bass/Tile).

Sharding: tensor-parallel over heads/groups. Core c owns group c =
16 heads (= 1024 gate/x channels, 128 B + 128 C state channels, 16 dt).
in_proj rows and out_proj columns are sharded accordingly; out_proj is
row-parallel over the contraction, partials are combined on the host.

v3: both projections run as fp8(e4m3) DoubleRow matmuls (K=256 per
pass, 0.5 cyc/row) with a 3-pass hi/lo correction that reuses tensors:
  PSUM = W8 x8 + W8 xlo + Wr x8  ~=  64 W x   (W8=e4m3(64W),
  Wr=e4m3(64W-W8), x8=e4m3(x), xlo=e4m3(x-x8))
The 1/64 is folded into consumers (conv weights, silu scale, dt
activation scale, host-side gather). Gate tiles are produced in
transposed [seq, chan] orientation straight into silu (drops the
per-chunk gate transposes); per-head cs broadcast is a single fp32r
matmul per head (free dim 256); out_proj moving operand is converted
to fp8 hi/lo on-chip right after the norm transposes.
"""

import numpy as np
import ml_dtypes

import concourse.bass as bass
import concourse.mybir as mybir
from concourse import bacc
from concourse.tile import TileContext
from concourse.bass_utils import run_bass_kernel_spmd

F32 = mybir.dt.float32
F32R = mybir.dt.float32r
BF16 = mybir.dt.bfloat16
F8 = mybir.dt.float8e4
AF = mybir.ActivationFunctionType
ALU = mybir.AluOpType
NPBF16 = ml_dtypes.bfloat16
NPF8 = ml_dtypes.float8_e4m3

# Model dims
H_SIZE = 4096
NH = 128
HD = 64
SS = 128
KCONV = 4
NG = 8
CHUNK = 128
INTER = NH * HD                 # 8192
CONV_DIM = INTER + 2 * NG * SS  # 10240
PROJ = INTER + CONV_DIM + NH    # 18560
DT_MIN, DT_MAX = 0.001, 100.0
EPS = 1e-5
GROUP = INTER // NG             # 1024

# Sharding / tiling
N_CORES = 8
S = 2048
HL = NH // N_CORES              # 16 local heads
CLOC = HL * HD                  # 1024 local gate/x channels
NSB = 4                         # seq superblocks
SB = S // NSB                   # 512
NCPB = SB // CHUNK              # 4 chunks per superblock
NKB = H_SIZE // 256             # 16 fp8 K-blocks for in_proj
NT = 18                         # w1 tiles: 0-7 x, 8 B, 9 C, 10-17 gate
NK2 = CLOC // 128               # 8 k-tiles for out_proj
NB2 = CLOC // 256               # 4 fp8 K-blocks for out_proj
NM2 = H_SIZE // 128             # 32 m-tiles for out_proj
WS = 64.0                       # fp8 weight prescale

_CACHE = {}


def r32(ap):
    return ap.bitcast(F32R)


def ap3(t, col, s1, n1, s2, n2):
    """3-dim AP into tile t at free offset col: [part][s1,n1][s2,n2]."""
    base = t[:, col:col + 1]
    return bass.AP(tensor=t.tensor, offset=base.offset,
                   ap=[base.ap[0], [s1, n1], [s2, n2]])


def _steer_act_tables():
    """Make the table-load placement pass choose the joint exp+ln table.

    The pass picks the first act-func-set containing each required
    function; by default Exp resolves to exp_and_others and Ln to
    natural_log, which thrashes a table load on every Exp<->Ln switch
    (dt pipeline, per-chunk rmsnorm). Hiding exp/ln from the
    single-function sets steers both to natural_log_exp_and_others.
    Set ids (= act_info.json indices) are untouched, so the runtime
    tables walrus emits stay valid.
    """
    if getattr(bacc, "_act_tables_steered", False):
        return
    orig = bacc.get_activation_tables

    def patched(arch):
        t = dict(orig(arch))
        for name in list(t):
            funcs = set(t[name])
            if name == "exp_and_others":
                funcs.discard(AF.Exp)
            if name == "natural_log":
                funcs.discard(AF.Ln)
            t[name] = funcs
        return t

    bacc.get_activation_tables = patched
    bacc._act_tables_steered = True


def build_nc():
    _steer_act_tables()
    nc = bacc.Bacc(None, target_bir_lowering=False)

    # fp8 hidden, quarter-split: [sb, q, 128, 4*1024] where block b=4q+bq
    # holds [2 ktiles x 512 seq] at col bq*1024+i*512+s
    hid8 = nc.declare_dram_parameter("hid8", [NSB, 4, 128, 4096], F8,
                                     isOutput=False)
    hidlo = nc.declare_dram_parameter("hidlo", [NSB, 4, 128, 4096], F8,
                                      isOutput=False)
    # in_proj weights per tile: [t, 128, b*256 + i*128 + m] fp8
    w18 = nc.declare_dram_parameter("w18", [NT, 128, NKB * 256], F8,
                                    isOutput=False)
    w1r = nc.declare_dram_parameter("w1r", [NT, 128, NKB * 256], F8,
                                    isOutput=False)
    # dt rows (16) ride the transposed gate path: [128, b*32 + i*16 + m]
    w1dt8 = nc.declare_dram_parameter("w1dt8", [128, NKB * 32], F8,
                                      isOutput=False)
    w1dtr = nc.declare_dram_parameter("w1dtr", [128, NKB * 32], F8,
                                      isOutput=False)
    # out_proj weights, groups of 4 m-tiles: [G, 128, g*1024+B*256+i*128+m]
    w28 = nc.declare_dram_parameter("w28", [NM2 // 4, 128, 4096], F8,
                                    isOutput=False)
    w2r = nc.declare_dram_parameter("w2r", [NM2 // 4, 128, 4096], F8,
                                    isOutput=False)
    convw = nc.declare_dram_parameter("convw", [128, 10 * KCONV], F32,
                                      isOutput=False)
    convb = nc.declare_dram_parameter("convb", [128, 10], F32, isOutput=False)
    dtbias = nc.declare_dram_parameter("dtbias", [HL, 1], F32, isOutput=False)
    acol = nc.declare_dram_parameter("acol", [HL, 1], F32, isOutput=False)
    dbc = nc.declare_dram_parameter("dbc", [128, HL], F32, isOutput=False)
    idf = nc.declare_dram_parameter("idf", [128, 128], F32, isOutput=False)
    idb = nc.declare_dram_parameter("idb", [128, 128], BF16, isOutput=False)
    trim = nc.declare_dram_parameter("trim", [128, 128], BF16, isOutput=False)
    # output partials at WS scale (host divides by WS)
    outp = nc.declare_dram_parameter("outp", [NM2, NSB, 128, SB], BF16,
                                     isOutput=True)

    with TileContext(nc) as tc:
        with tc.tile_pool(name="const", bufs=1) as cp:
            idf_sb = cp.tile([128, 128], F32, tag="idf")
            idb_sb = cp.tile([128, 128], BF16, tag="idb")
            trim_sb = cp.tile([128, 128], BF16, tag="trim")
            cw_sb = cp.tile([128, 10 * KCONV], F32, tag="cw")
            cb_sb = cp.tile([128, 10], F32, tag="cb")
            dtb_sb = cp.tile([HL, 1], F32, tag="dtb")
            a_sb = cp.tile([HL, 1], F32, tag="acol")
            dbc_sb = cp.tile([128, HL], F32, tag="dbc")
            ones16 = cp.tile([HL, 1], F32, tag="ones16")
            zcol = cp.tile([128, 1], F32, tag="zcol")
            st_sb = cp.tile([128, HL * HD], F32R, tag="state")
            stT = cp.tile([128, HL * HD], BF16, tag="stateb")
            dt8_sb = cp.tile([128, NKB * 32], F8, tag="dt8")
            dtr_sb = cp.tile([128, NKB * 32], F8, tag="dtr")
            nc.vector.memset(ones16[:], 1.0)
            nc.vector.memset(zcol[:], 0.0)

            # const DMAs are issued inside _main_phase after sb0's
            # critical-path fp8 loads (9 small DMAs would otherwise delay
            # the first in_proj tile by several us)
            def load_consts():
                nc.sync.dma_start(out=dt8_sb[:], in_=w1dt8[:])
                nc.sync.dma_start(out=dtr_sb[:], in_=w1dtr[:])
                nc.sync.dma_start(out=dtb_sb[:], in_=dtbias[:])
                nc.sync.dma_start(out=a_sb[:], in_=acol[:])
                nc.sync.dma_start(out=idf_sb[:], in_=idf[:])
                nc.sync.dma_start(out=cw_sb[:], in_=convw[:])
                nc.sync.dma_start(out=cb_sb[:], in_=convb[:])
                nc.sync.dma_start(out=idb_sb[:], in_=idb[:])
                nc.sync.dma_start(out=trim_sb[:], in_=trim[:])
                nc.sync.dma_start(out=dbc_sb[:], in_=dbc[:])

            _main_phase(nc, tc, load_consts, hid8, hidlo, w18, w1r, w28, w2r,
                        outp, idf_sb, idb_sb, trim_sb, cw_sb, cb_sb,
                        dtb_sb, a_sb, dbc_sb, ones16, zcol, st_sb, stT,
                        dt8_sb, dtr_sb)

    nc.compile()
    return nc


def _main_phase(nc, tc, load_consts, hid8, hidlo, w18, w1r, w28, w2r, outp,
                idf_sb, idb_sb, trim_sb, cw_sb, cb_sb,
                dtb_sb, a_sb, dbc_sb, ones16, zcol, st_sb, stT,
                dt8_sb, dtr_sb):
    with tc.tile_pool(name="hid", bufs=1) as hidp, \
         tc.tile_pool(name="w1", bufs=3) as w1p, \
         tc.tile_pool(name="w2", bufs=2) as w2p, \
         tc.tile_pool(name="stage", bufs=1) as sgp, \
         tc.tile_pool(name="qst", bufs=2) as qstp, \
         tc.tile_pool(name="conv32", bufs=2) as cvp, \
         tc.tile_pool(name="pair", bufs=1) as prp, \
         tc.tile_pool(name="seg", bufs=2) as segp, \
         tc.tile_pool(name="ch", bufs=2) as chp, \
         tc.tile_pool(name="ch1", bufs=1) as ch1p, \
         tc.tile_pool(name="oev", bufs=3) as oevp, \
         tc.tile_pool(name="acc", bufs=2, space="PSUM") as accp, \
         tc.tile_pool(name="psY", bufs=2, space="PSUM") as psY, \
         tc.tile_pool(name="psPB", bufs=2, space="PSUM") as psPB, \
         tc.tile_pool(name="psT", bufs=2, space="PSUM") as psT:

        # conv input staging: 10 channel tiles (8 x, 1 B, 1 C), 3 halo + SB
        # values are at WS scale; conv weights are pre-divided by WS.
        ccat = sgp.tile([128, 10 * (SB + 3)], F32, tag="ccat")
        for t in range(10):
            nc.vector.memset(ccat[:, t * (SB + 3):t * (SB + 3) + 3], 0.0)

        pending_out = []

        w2cache = {}

        def load_w2_pair(G, name="w2"):
            t8 = w2p.tile([128, 4096], F8, tag="w28", name=name + "8")
            tr = w2p.tile([128, 4096], F8, tag="w2r", name=name + "r")
            nc.sync.dma_start(out=t8[:], in_=w28[G])
            nc.sync.dma_start(out=tr[:], in_=w2r[G])
            return (t8, tr)

        def emit_outproj(m, qpair, sbq, pool=None, tag="acc"):
            q8t, qlot = qpair
            G, g = m // 4, m % 4
            if w2cache.get("G") != G:
                pair = (w2cache.get("pref")
                        if w2cache.get("prefG") == G else None)
                if pair is None:
                    pair = load_w2_pair(G)
                w2cache["G"] = G
                w2cache["t"] = pair
                w2cache["pref"] = None
            w2t8, w2tr = w2cache["t"]
            if g == 0 and G + 1 < NM2 // 4 and w2cache.get("prefG") != G + 1:
                w2cache["prefG"] = G + 1
                w2cache["pref"] = load_w2_pair(G + 1, name="w2pref")
            acc = (pool or accp).tile([128, SB], F32, tag=tag)
            for h in range(2):
                out = acc[:, h * 256:(h + 1) * 256]
                n = 0
                for B in range(NB2):
                    st8 = ap3(w2t8, g * 1024 + B * 256, 128, 2, 1, 128)
                    str_ = ap3(w2tr, g * 1024 + B * 256, 128, 2, 1, 128)
                    mv8 = ap3(q8t, (2 * B) * SB + h * 256, SB, 2, 1, 256)
                    mvlo = ap3(qlot, (2 * B) * SB + h * 256, SB, 2, 1, 256)
                    for sta, mv in ((st8, mv8), (st8, mvlo), (str_, mv8)):
                        nc.tensor.matmul(
                            out, sta, mv,
                            start=(n == 0), stop=(n == 3 * NB2 - 1),
                            perf_mode=mybir.MatmulPerfMode.DoubleRow)
                        n += 1
            ev = oevp.tile([128, SB], BF16, tag="oev")
            nc.scalar.copy(ev[:], acc[:])
            nc.sync.dma_start(out=outp[m, sbq], in_=ev[:])

        for sb in range(NSB):
            # DMA order follows pass-major consumption: W8, x8 quarters,
            # Wr, then xlo quarters
            w1pre8 = w1p.tile([128, NKB * 256], F8, tag="w18", name="w1pre8")
            w1prer = w1p.tile([128, NKB * 256], F8, tag="w1r", name="w1prer")
            nc.sync.dma_start(out=w1pre8[:], in_=w18[8])
            h8q, hloq = [], []
            for q in range(4):
                t8 = hidp.tile([128, 4096], F8, tag=f"hid8{q}",
                               name=f"hid8{q}")
                nc.sync.dma_start(out=t8[:], in_=hid8[sb, q])
                h8q.append(t8)
            nc.sync.dma_start(out=w1prer[:], in_=w1r[8])
            for q in range(4):
                tlo = hidp.tile([128, 4096], F8, tag=f"hidlo{q}",
                                name=f"hidlo{q}")
                nc.sync.dma_start(out=tlo[:], in_=hidlo[sb, q])
                hloq.append(tlo)
            if sb == 0:
                load_consts()

            dtraw = sgp.tile([HL, SB], F32, tag="dtraw")
            silg_sb = sgp.tile([128, NCPB * CLOC], BF16, tag="silg_sb")

            # halo copies must read previous superblock before overwrite
            if sb > 0:
                for t in range(10):
                    base = t * (SB + 3)
                    nc.vector.tensor_copy(
                        ccat[:, base:base + 3], ccat[:, base + SB:base + SB + 3])

            # pass structure per K-block: (W8,x8), (W8,xlo), (Wr,x8)
            def emit_stationary_tile(t, dtraw=dtraw):
                if t == 8:
                    w8t, wrt = w1pre8, w1prer
                else:
                    w8t = w1p.tile([128, NKB * 256], F8, tag="w18")
                    wrt = w1p.tile([128, NKB * 256], F8, tag="w1r")
                    nc.sync.dma_start(out=w8t[:], in_=w18[t])
                    nc.sync.dma_start(out=wrt[:], in_=w1r[t])
                acc = accp.tile([128, SB], F32, tag="acc")
                for h in range(2):
                    out = acc[:, h * 256:(h + 1) * 256]
                    n = 0
                    # pass-major: (W8,x8), (Wr,x8), (W8,xlo) so the first
                    # passes only need the hi-tensor DMAs
                    for which in range(3):
                        for b in range(NKB):
                            q, qc = b // 4, (b % 4) * 1024 + h * 256
                            if which == 0:
                                sta = ap3(w8t, b * 256, 128, 2, 1, 128)
                                mv = ap3(h8q[q], qc, 512, 2, 1, 256)
                            elif which == 1:
                                sta = ap3(wrt, b * 256, 128, 2, 1, 128)
                                mv = ap3(h8q[q], qc, 512, 2, 1, 256)
                            else:
                                sta = ap3(w8t, b * 256, 128, 2, 1, 128)
                                mv = ap3(hloq[q], qc, 512, 2, 1, 256)
                            nc.tensor.matmul(
                                out, sta, mv,
                                start=(n == 0), stop=(n == 3 * NKB - 1),
                                perf_mode=mybir.MatmulPerfMode.DoubleRow)
                            n += 1
                base = t * (SB + 3)
                nc.scalar.copy(ccat[:, base + 3:base + 3 + SB], acc[:])

            def emit_gate_tile(tg, silg_sb=silg_sb):
                w8t = w1p.tile([128, NKB * 256], F8, tag="w18")
                wrt = w1p.tile([128, NKB * 256], F8, tag="w1r")
                nc.sync.dma_start(out=w8t[:], in_=w18[10 + tg])
                nc.sync.dma_start(out=wrt[:], in_=w1r[10 + tg])
                acc = accp.tile([128, SB], F32, tag="acc")
                for st in range(NCPB):
                    out = acc[:, st * 128:(st + 1) * 128]
                    n = 0
                    for b in range(NKB):
                        q, qc = b // 4, (b % 4) * 1024 + st * 128
                        lh8 = ap3(h8q[q], qc, 512, 2, 1, 128)
                        lhlo = ap3(hloq[q], qc, 512, 2, 1, 128)
                        mv8 = ap3(w8t, b * 256, 128, 2, 1, 128)
                        mvr = ap3(wrt, b * 256, 128, 2, 1, 128)
                        for sta, mv in ((lh8, mv8), (lhlo, mv8), (lh8, mvr)):
                            nc.tensor.matmul(
                                out, sta, mv,
                                start=(n == 0), stop=(n == 3 * NKB - 1),
                                perf_mode=mybir.MatmulPerfMode.DoubleRow)
                            n += 1
                # silu straight out of PSUM into [seq, chan] slab
                dst = ap3(silg_sb, tg * 128, CLOC, NCPB, 1, 128)
                src = ap3(acc, 0, 128, NCPB, 1, 128)
                nc.scalar.activation(dst, src, AF.Silu, scale=1.0 / WS)

            def emit_dt_gate(dtraw=dtraw):
                # dt's 16 rows as a narrow moving tile in the transposed
                # gate orientation: out free 16 costs 8 cyc/matmul vs the
                # old full 128-row stationary tile (128 cyc)
                acc = accp.tile([128, SB], F32, tag="acc")
                for st in range(NCPB):
                    out = acc[:, st * 16:(st + 1) * 16]
                    n = 0
                    for b in range(NKB):
                        q, qc = b // 4, (b % 4) * 1024 + st * 128
                        lh8 = ap3(h8q[q], qc, 512, 2, 1, 128)
                        lhlo = ap3(hloq[q], qc, 512, 2, 1, 128)
                        mv8 = ap3(dt8_sb, b * 32, 16, 2, 1, 16)
                        mvr = ap3(dtr_sb, b * 32, 16, 2, 1, 16)
                        for sta, mv in ((lh8, mv8), (lhlo, mv8), (lh8, mvr)):
                            nc.tensor.matmul(
                                out, sta, mv,
                                start=(n == 0), stop=(n == 3 * NKB - 1),
                                perf_mode=mybir.MatmulPerfMode.DoubleRow)
                            n += 1
                # [seq, 16] -> [16, seq] via 4 transposes
                dtg = chp.tile([128, 4 * 16], F32, tag="dtg")
                nc.scalar.copy(dtg[:], acc[:, :4 * 16])
                pdt = psPB.tile([128, 512], F32, tag="pb", name="pdt")
                for st in range(NCPB):
                    nc.tensor.transpose(
                        pdt[:HL, st * 128:(st + 1) * 128],
                        dtg[:, st * 16:(st + 1) * 16], idf_sb[:])
                nc.scalar.copy(dtraw[:, :], pdt[:HL, :])

            def emit_dt_pipeline():
                az = sgp.tile([HL, SB], F32, tag="az")
                dtsp = dtraw  # in-place: relu(z)+ln1p overwrites raw dt
                nc.scalar.activation(az[:], dtraw[:], AF.Abs,
                                     bias=dtb_sb[:, 0:1], scale=1.0 / WS)
                nc.scalar.activation(az[:], az[:], AF.Exp, scale=-1.0)
                nc.vector.tensor_scalar(az[:], az[:], 1.0, None, ALU.add)
                nc.scalar.activation(az[:], az[:], AF.Ln)
                nc.scalar.activation(dtsp[:], dtraw[:], AF.Relu,
                                     bias=dtb_sb[:, 0:1], scale=1.0 / WS)
                nc.vector.tensor_tensor(dtsp[:], dtsp[:], az[:], ALU.add)
                nc.vector.tensor_scalar(dtsp[:], dtsp[:], DT_MIN, DT_MAX,
                                        ALU.max, ALU.min)
                dA = az  # az dead, reuse
                nc.vector.tensor_scalar(dA[:], dtsp[:], a_sb[:, 0:1], None,
                                        ALU.mult)
                for cl in range(NCPB):
                    ones_b = bass.AP(tensor=ones16.tensor,
                                     offset=ones16[:].offset,
                                     ap=[ones16[:].ap[0], [0, CHUNK]])
                    nc.vector.tensor_tensor_scan(
                        cs[:, cl * CHUNK:(cl + 1) * CHUNK],
                        ones_b, dA[:, cl * CHUNK:(cl + 1) * CHUNK],
                        0.0, ALU.mult, ALU.add)
                return dtsp

            cs = sgp.tile([HL, SB], F32, tag="cs")
            drain = list(pending_out)
            pending_out.clear()
            emit_stationary_tile(8)
            emit_dt_gate()
            dtsp = emit_dt_pipeline()
            for u in [9] + list(range(8)):
                emit_stationary_tile(u)

            # csT/dtT for all chunks: [128, cl*HL + h] / [128, (4+cl)*HL + h]
            pcs = psPB.tile([128, 2 * NCPB * HL], F32, tag="pb",
                            name="pcs")
            for cl in range(NCPB):
                nc.tensor.transpose(
                    pcs[:, cl * HL:(cl + 1) * HL],
                    cs[:, cl * CHUNK:(cl + 1) * CHUNK], idf_sb[:HL, :HL])
                nc.tensor.transpose(
                    pcs[:, (NCPB + cl) * HL:(NCPB + cl + 1) * HL],
                    dtsp[:, cl * CHUNK:(cl + 1) * CHUNK], idf_sb[:HL, :HL])
            csdtT = sgp.tile([128, 2 * NCPB * HL], F32, tag="csdtT")
            nc.scalar.copy(csdtT[:], pcs[:])
            negcsT = sgp.tile([128, NCPB * HL], F32, tag="negcsT")
            nc.vector.tensor_scalar(negcsT[:], csdtT[:, :NCPB * HL], -1.0,
                                    None, ALU.mult)

            # bf16 triple splits of cs per pair: the broadcast matmuls need
            # ~21 bits of cs mantissa (fp32r/tf32 is NOT enough: |cs| can be
            # ~2e4 while exp needs abs err << 0.01)
            splits = []
            for pr2 in range(NCPB // 2):
                p2sl = slice(pr2 * 2 * CHUNK, (pr2 + 1) * 2 * CHUNK)
                csh = sgp.tile([HL, 2 * CHUNK], BF16, tag="csh", bufs=2,
                               name=f"csh{pr2}")
                csm = sgp.tile([HL, 2 * CHUNK], BF16, tag="csm", bufs=2,
                               name=f"csm{pr2}")
                csl_ = sgp.tile([HL, 2 * CHUNK], BF16, tag="csl", bufs=2,
                               name=f"csl{pr2}")
                res = sgp.tile([HL, 2 * CHUNK], F32, tag="csres", bufs=2,
                               name=f"res{pr2}")
                nc.vector.tensor_copy(csh[:], cs[:, p2sl])
                nc.vector.tensor_tensor(res[:], cs[:, p2sl], csh[:],
                                        ALU.subtract)
                nc.vector.tensor_copy(csm[:], res[:])
                nc.vector.tensor_tensor(res[:], res[:], csm[:], ALU.subtract)
                nc.vector.tensor_copy(csl_[:], res[:])
                splits.append((csh, csm, csl_))

            # conv (DVE-heavy) emitted before gate tiles (PE-heavy) so the
            # two engines overlap; conv + gate silus share one table window
            xcs = sgp.tile([128, 8 * SB], BF16, tag="xcs")
            bcs = sgp.tile([128, SB], BF16, tag="bcs")
            ccs = sgp.tile([128, SB], BF16, tag="ccs")
            for t in [8, 9] + list(range(8)):
                base = t * (SB + 3)
                eng = nc.vector
                c32 = cvp.tile([128, SB], F32, tag="c32")
                eng.tensor_scalar(
                    c32[:], ccat[:, base:base + SB],
                    cw_sb[:, t * KCONV:t * KCONV + 1], cb_sb[:, t:t + 1],
                    ALU.mult, ALU.add)
                for j in range(1, KCONV):
                    eng.scalar_tensor_tensor(
                        c32[:], ccat[:, base + j:base + j + SB],
                        cw_sb[:, t * KCONV + j:t * KCONV + j + 1], c32[:],
                        ALU.mult, ALU.add)
                dst = (xcs[:, t * SB:(t + 1) * SB] if t < 8
                       else (bcs[:] if t == 8 else ccs[:]))
                nc.scalar.activation(dst, c32[:], AF.Silu)

            for tg in range(8):
                emit_gate_tile(tg)

            q8t = qstp.tile([128, NK2 * SB], F8, tag="q8")
            qlot = qstp.tile([128, NK2 * SB], F8, tag="qlo")
            ssum = sgp.tile([128, NCPB], F32, tag="ssum")

            # ---------------- SSD chunk pairs ----------------
            for pr in range(NCPB // 2):
                prsl = slice(pr * 2 * CHUNK, (pr + 1) * 2 * CHUNK)
                csh, csm, csl_ = splits[pr]
                # per-head cs broadcast: pb[p, j*256+l] = cs[h, pr*256+l]
                epb = prp.tile([128, HL * 2 * CHUNK], F32R, tag="epb")
                segs = [segp.tile([128, HL * CHUNK], F32R, tag="seg",
                                  name=f"seg{i}")
                        for i in range(2)]
                for hg in range(HL // 2):
                    # PE filler: the pb->seg->exp chain is Act/DVE-bound
                    if hg % 2 == 1 and drain:
                        emit_outproj(*drain.pop(0))
                    pb = psPB.tile([128, 512], F32, tag="pb")
                    for j in range(2):
                        h = 2 * hg + j
                        idcol = idb_sb[:HL, h:h + 1]
                        indh = bass.AP(tensor=idcol.tensor,
                                       offset=idcol.offset,
                                       ap=[[idcol.ap[0][0], HL], [0, 128]])
                        for si, spl in enumerate((csh, csm, csl_)):
                            nc.tensor.matmul(pb[:, j * 256:(j + 1) * 256],
                                             indh, spl[:],
                                             start=(si == 0), stop=(si == 2))
                    # seg[s, l] = min(cs[h,l] - cs[h,s], 0) per chunk
                    for lc in range(2):
                        cl = 2 * pr + lc
                        for j in range(2):
                            h = 2 * hg + j
                            nc.vector.scalar_tensor_tensor(
                                segs[lc][:, h * CHUNK:(h + 1) * CHUNK],
                                pb[:, j * 256 + lc * 128:
                                   j * 256 + (lc + 1) * 128],
                                negcsT[:, cl * HL + h:cl * HL + h + 1],
                                bass.AP(tensor=zcol.tensor,
                                        offset=zcol[:].offset,
                                        ap=[zcol[:].ap[0], [0, CHUNK]]),
                                ALU.add, ALU.min)
                    nc.scalar.activation(epb[:, hg * 512:(hg + 1) * 512],
                                         pb[:], AF.Exp)

                for lc in range(2):
                    cl = 2 * pr + lc
                    def filler(n, drain=drain):
                        for _ in range(min(n, len(drain))):
                            emit_outproj(*drain.pop(0))
                    _emit_chunk(nc, sb * NCPB + cl, cl, lc, silg_sb, xcs,
                                bcs, ccs,
                                csdtT, segs[lc], epb, q8t, qlot, ssum,
                                idf_sb, idb_sb, trim_sb, dbc_sb,
                                st_sb, stT, chp, ch1p, psY, psT, psPB,
                                filler)

            while drain:
                emit_outproj(*drain.pop(0))

            pending_out.extend((m, (q8t, qlot), sb) for m in range(NM2))

        # final drain: rotate across all psum pools so the ev-copy WAR
        # latency of one bank hides behind matmuls into another
        pools = [(accp, "acc"), (psY, "y"), (psPB, "pb")]
        i = 0
        while pending_out:
            pool, tag = pools[i % 3]
            i += 1
            emit_outproj(*pending_out.pop(0), pool=pool, tag=tag)


def _emit_chunk(nc, gc, cl, lc, silg_sb, xcs, bcs, ccs,
                csdtT, seg, epb, q8t, qlot, ssum,
                idf_sb, idb_sb, trim_sb, dbc_sb,
                st_sb, stT, chp, ch1p, psY, psT, psPB, filler):
    csl = slice(cl * CHUNK, (cl + 1) * CHUNK)

    # gate already in [seq, chan] with silu applied
    silg = silg_sb[:, cl * CLOC:(cl + 1) * CLOC]

    # scores = exp(seg) * (triu-in-[s,l] . gram); gram^T = B C^T in [s, l]
    gram_ps = psPB.tile([128, 128], F32, tag="pb", name="gram_ps")
    nc.tensor.matmul(gram_ps[:], bcs[:, csl], ccs[:, csl],
                     start=True, stop=True)
    gram = chp.tile([128, 128], F32, tag="gramm")
    nc.vector.tensor_tensor(gram[:], gram_ps[:], trim_sb[:], ALU.mult)

    # chunk-end decay per head: cend = exp(cs_end), decT = exp(cs_end - cs)
    # (both extracted BEFORE seg/epb are overwritten in place below)
    cend = chp.tile([128, HL], F32, tag="cend")
    ep1 = epb[:, (lc + 1) * CHUNK - 1:(lc + 1) * CHUNK]
    epb_end = bass.AP(tensor=epb.tensor, offset=ep1.offset,
                      ap=[ep1.ap[0], [2 * CHUNK, HL]])
    nc.vector.tensor_copy(cend[:], epb_end)
    decT = chp.tile([128, HL], F32, tag="decT")
    # seg column l=CHUNK-1 holds cs_end - cs[s] (<=0, min-clamp no-op there)
    sg1 = seg[:, CHUNK - 1:CHUNK]
    seg_end = bass.AP(tensor=seg.tensor, offset=sg1.offset,
                      ap=[sg1.ap[0], [CHUNK, HL]])
    nc.scalar.activation(decT[:], seg_end, AF.Exp)

    # scores = exp(seg) * gram -> bf16
    scores = chp.tile([128, HL * CHUNK], BF16, tag="scores", bufs=1)
    nc.scalar.activation(scores[:], seg[:], AF.Exp)
    s3 = scores[:].rearrange("p (h l) -> p h l", h=HL)
    gram_b = bass.AP(tensor=gram.tensor, offset=gram[:].offset,
                     ap=[gram[:].ap[0], [0, HL], [1, 128]])
    nc.vector.tensor_tensor(s3, s3, gram_b, ALU.mult)

    # e4 = exp(pb) * C (for Yoff) -> bf16
    e4 = chp.tile([128, HL * CHUNK], BF16, tag="e4", bufs=1)
    e4_3 = e4[:].rearrange("p (h l) -> p h l", h=HL)
    ep0 = epb[:, lc * CHUNK:lc * CHUNK + 1]
    epb_3 = bass.AP(tensor=epb.tensor, offset=ep0.offset,
                    ap=[ep0.ap[0], [2 * CHUNK, HL], [1, CHUNK]])
    cc0 = ccs[:, cl * CHUNK:cl * CHUNK + 1]
    ccs_b = bass.AP(tensor=ccs.tensor, offset=cc0.offset,
                    ap=[cc0.ap[0], [0, HL], [1, CHUNK]])
    nc.vector.tensor_tensor(e4_3, epb_3, ccs_b, ALU.mult)
    ddt = chp.tile([128, HL], F32, tag="ddt")
    nc.vector.tensor_tensor(ddt[:], csdtT[:, (NCPB + cl) * HL:
                                           (NCPB + cl + 1) * HL],
                            decT[:], ALU.mult)

    # x transpose -> xT (bf16), then xdt / xdd
    xT = ch1p.tile([128, CLOC], BF16, tag="xT")
    for hx in range(2):
        xps = psT.tile([128, 512], BF16, tag="trans", name=f"xps{hx}")
        for t in range(4):
            tt = hx * 4 + t
            nc.tensor.transpose(
                xps[:, t * 128:(t + 1) * 128],
                xcs[:, tt * SB + cl * CHUNK:tt * SB + (cl + 1) * CHUNK],
                idb_sb[:])
        nc.scalar.copy(xT[:, hx * 512:(hx + 1) * 512], xps[:])
    xdt = ch1p.tile([128, CLOC], BF16, tag="xdt")
    x3 = xT[:].rearrange("p (h j) -> p h j", h=HL)
    dt0 = csdtT[:, (NCPB + cl) * HL:(NCPB + cl) * HL + 1]
    dt_b = bass.AP(tensor=csdtT.tensor, offset=dt0.offset,
                   ap=[dt0.ap[0], [1, HL], [0, HD]])
    ddt_b = bass.AP(tensor=ddt.tensor, offset=ddt[:].offset,
                    ap=[ddt[:].ap[0], [1, HL], [0, HD]])
    nc.vector.tensor_tensor(xdt[:].rearrange("p (h j) -> p h j", h=HL),
                            x3, dt_b, ALU.mult)
    # ysb = x*D now (before xdd overwrites xT in place)
    dbc_b = bass.AP(tensor=dbc_sb.tensor, offset=dbc_sb[:].offset,
                    ap=[dbc_sb[:].ap[0], [1, HL], [0, HD]])
    ysb = ch1p.tile([128, CLOC], F32, tag="ysb")
    nc.vector.tensor_tensor(ysb[:].rearrange("p (h j) -> p h j", h=HL),
                            x3, dbc_b, ALU.mult)
    xdd = xT  # in place: x * ddt overwrites xT
    nc.vector.tensor_tensor(xdd[:].rearrange("p (h j) -> p h j", h=HL),
                            x3, ddt_b, ALU.mult)

    # B chunk transposed (bln) for state matmuls
    pbt = psPB.tile([128, 128], BF16, tag="pb", name="pbt")
    nc.tensor.transpose(pbt[:], bcs[:, csl], idb_sb[:])
    bln = chp.tile([128, 128], BF16, tag="bln")
    nc.scalar.copy(bln[:], pbt[:])

    filler(3)

    # Ydiag + Yoff accumulated per head (two matmuls per head)
    y_halves = []
    for half in range(2):
        y_ps = psY.tile([128, 512], F32, tag="y", name=f"y{half}")
        for hh in range(8):
            h = half * 8 + hh
            hs = slice(hh * HD, (hh + 1) * HD)
            nc.tensor.matmul(
                y_ps[:, hs], scores[:, h * CHUNK:(h + 1) * CHUNK],
                xdt[:, h * HD:(h + 1) * HD], start=True, stop=(gc == 0))
            if gc > 0:
                nc.tensor.matmul(
                    y_ps[:, hs],
                    e4[:, h * CHUNK:(h + 1) * CHUNK],
                    stT[:, h * HD:(h + 1) * HD], start=False, stop=True)
        y_halves.append(y_ps)

    # states for this chunk
    s_halves = []
    for half in range(2):
        s_ps = psY.tile([128, 512], F32, tag="y", name=f"s{half}")
        nc.tensor.matmul(
            s_ps[:], bln[:], xdd[:, half * 512:(half + 1) * 512],
            start=True, stop=True)
        s_halves.append(s_ps)

    filler(5)

    # y = (Ydiag + Yoff) + D*x, gated; squares accumulated for RMS
    for half in range(2):
        hsl = slice(half * 512, (half + 1) * 512)
        nc.vector.tensor_tensor(ysb[:, hsl], ysb[:, hsl],
                                y_halves[half][:], ALU.add)
    nc.vector.tensor_tensor(ysb[:], ysb[:], silg, ALU.mult)
    nc.scalar.activation(xdt[:], ysb[:], AF.Square,
                         accum_out=ssum[:, cl:cl + 1])

    # per-chunk group RMSNorm + transpose + fp8 hi/lo conversion
    lnm = chp.tile([128, 1], F32, tag="lnm")
    rstd = chp.tile([128, 1], F32, tag="rstd")
    nc.vector.tensor_scalar(lnm[:], ssum[:, cl:cl + 1], 1.0 / GROUP, EPS,
                            ALU.mult, ALU.add)
    nc.scalar.activation(lnm[:], lnm[:], AF.Ln)
    nc.scalar.activation(rstd[:], lnm[:], AF.Exp, scale=-0.5)
    normed = ch1p.tile([128, CLOC], BF16, tag="normed")
    nc.vector.tensor_scalar(normed[:], ysb[:], rstd[:, 0:1], None, ALU.mult)
    nps = psT.tile([128, CLOC], BF16, tag="trans")
    for t in range(NK2):
        nc.tensor.transpose(
            nps[:, t * 128:(t + 1) * 128],
            normed[:, t * 128:(t + 1) * 128], idb_sb[:])
    nsrc = nps[:].rearrange("p (t s) -> p t s", t=NK2)
    q8dst = ap3(q8t, cl * 128, SB, NK2, 1, 128)
    qlodst = ap3(qlot, cl * 128, SB, NK2, 1, 128)
    nc.scalar.copy(q8dst, nsrc)
    nc.vector.tensor_tensor(qlodst, nsrc, q8dst, ALU.subtract)

    # state update: st = st * exp(cs_end) + s  (first chunk: st = s)
    if gc == 0:
        for half in range(2):
            hsl = slice(half * 512, (half + 1) * 512)
            nc.vector.tensor_copy(st_sb[:, hsl], s_halves[half][:])
        nc.vector.tensor_copy(stT[:], st_sb[:])
    else:
        cend_b = bass.AP(tensor=cend.tensor, offset=cend[:].offset,
                         ap=[cend[:].ap[0], [1, HL], [0, HD]])
        st3 = st_sb[:].rearrange("p (h j) -> p h j", h=HL)
        nc.vector.tensor_tensor(st3, st3, cend_b, ALU.mult)
        for half in range(2):
            hsl = slice(half * 512, (half + 1) * 512)
            nc.vector.tensor_tensor(st_sb[:, hsl], st_sb[:, hsl],
                                    s_halves[half][:], ALU.add)
        nc.vector.tensor_copy(stT[:], st_sb[:])


def _q8(a):
    return np.asarray(a, NPF8)


def prepare_in_maps(hidden_states, in_proj_w, conv_w, conv_b, dt_bias, D,
                    norm_w, out_proj_w):
    hidT = np.ascontiguousarray(
        hidden_states.reshape(S, H_SIZE).T).astype(np.float32)
    x8 = _q8(hidT)
    xlo = _q8(hidT - x8.astype(np.float32))

    def hid_layout(a):
        # [k=4096, s=2048] -> [sb, q, p, bq*1024 + i*512 + s]
        return np.ascontiguousarray(
            a.reshape(4, 4, 2, 128, NSB, 512)
            .transpose(4, 0, 3, 1, 2, 5).reshape(NSB, 4, 128, 4096))

    idf = np.eye(128, dtype=np.float32)
    idb = np.eye(128).astype(NPBF16)
    # mask in [s, l]: keep l >= s
    trim = np.triu(np.ones((128, 128), np.float32)).astype(NPBF16)
    in_maps = []
    for c in range(N_CORES):
        gsl = slice(CLOC * c, CLOC * (c + 1))
        xsl = slice(INTER + CLOC * c, INTER + CLOC * (c + 1))
        bsl = slice(2 * INTER + SS * c, 2 * INTER + SS * (c + 1))
        cslc = slice(2 * INTER + NG * SS + SS * c,
                     2 * INTER + NG * SS + SS * (c + 1))
        dsl = slice(INTER + CONV_DIM + HL * c, INTER + CONV_DIM + HL * (c + 1))
        # tiles 0-7 x, 8 B, 9 C, 10-17 gate; dt rides separately
        w1 = np.concatenate([in_proj_w[xsl], in_proj_w[bsl], in_proj_w[cslc],
                             in_proj_w[gsl]], axis=0) * WS
        w1_8 = _q8(w1)
        w1_r = _q8(w1 - w1_8.astype(np.float32))

        def w1_layout(a):
            # [t*128+m, k=256b+128i+p] -> [t, p, b*256 + i*128 + m]
            return np.ascontiguousarray(
                a.reshape(NT, 128, NKB, 2, 128)
                .transpose(0, 4, 2, 3, 1).reshape(NT, 128, NKB * 256))

        dtw = in_proj_w[dsl] * WS                     # [16, 4096]
        dt_8 = _q8(dtw)
        dt_r = _q8(dtw - dt_8.astype(np.float32))

        def dt_layout(a):
            # [m=16, k=256b+128i+p] -> [p, b*32 + i*16 + m]
            return np.ascontiguousarray(
                a.reshape(16, NKB, 2, 128)
                .transpose(3, 1, 2, 0).reshape(128, NKB * 32))

        w2 = (out_proj_w[:, gsl] * norm_w[gsl][None, :]) * WS
        w2_8 = _q8(w2)
        w2_r = _q8(w2 - w2_8.astype(np.float32))

        def w2_layout(a):
            # [(4G+g)*128+m, c=256B+128i+p] -> [G, p, g*1024+B*256+i*128+m]
            return np.ascontiguousarray(
                a.reshape(8, 4, 128, NB2, 2, 128)
                .transpose(0, 5, 1, 3, 4, 2).reshape(8, 128, 4096))

        conv_idx = np.concatenate([
            np.arange(CLOC * c, CLOC * (c + 1)),
            np.arange(INTER + SS * c, INTER + SS * (c + 1)),
            np.arange(INTER + NG * SS + SS * c,
                      INTER + NG * SS + SS * (c + 1))])
        cwl = conv_w[conv_idx, 0, :] / WS     # [1280, 4], WS folded
        cbl = conv_b[conv_idx]                # [1280]
        convw = np.ascontiguousarray(
            cwl.reshape(10, 128, KCONV).transpose(1, 0, 2)
            .reshape(128, 10 * KCONV)).astype(np.float32)
        convb = np.ascontiguousarray(
            cbl.reshape(10, 128).transpose(1, 0)).astype(np.float32)
        hsl = slice(HL * c, HL * (c + 1))
        acol = -(np.arange(HL * c + 1, HL * (c + 1) + 1, dtype=np.float32))
        in_maps.append({
            "hid8": hid_layout(x8),
            "hidlo": hid_layout(xlo),
            "w18": w1_layout(w1_8),
            "w1r": w1_layout(w1_r),
            "w1dt8": dt_layout(dt_8),
            "w1dtr": dt_layout(dt_r),
            "w28": w2_layout(w2_8),
            "w2r": w2_layout(w2_r),
            "convw": convw,
            "convb": convb,
            "dtbias": dt_bias[hsl].reshape(HL, 1).astype(np.float32),
            "acol": acol.reshape(HL, 1),
            "dbc": np.tile(D[hsl][None, :], (128, 1)).astype(np.float32),
            "idf": idf,
            "idb": idb,
            "trim": trim,
        })
    return in_maps


def get_nc():
    if "nc" not in _CACHE:
        _CACHE["nc"] = build_nc()
    return _CACHE["nc"]


def kernel(hidden_states, in_proj_w, conv_w, conv_b, dt_bias, D, norm_w,
           out_proj_w):
    nc = get_nc()
    in_maps = prepare_in_maps(
        np.asarray(hidden_states, np.float32),
        np.asarray(in_proj_w, np.float32),
        np.asarray(conv_w, np.float32), np.asarray(conv_b, np.float32),
        np.asarray(dt_bias, np.float32), np.asarray(D, np.float32),
        np.asarray(norm_w, np.float32), np.asarray(out_proj_w, np.float32))
    res = run_bass_kernel_spmd(nc, in_maps, list(range(N_CORES)))
    acc = np.zeros((H_SIZE, S), np.float64)
    for r in res.results:
        acc += np.asarray(r["outp"], np.float64).transpose(0, 2, 1, 3) \
                 .reshape(H_SIZE, S)
    return (acc / WS).T.astype(np.float32).reshape(1, S, H_SIZE)
